# revision 61
# baseline (speedup 1.0000x reference)
"""Causal self-attention (B=2, S=2048, D=1024, H=16) on 8 Trainium2 NeuronCores.

Sharding: core c handles batch b = c//4 and head-group g = c%4 (4 heads, 256
channels); compute is bf16 with fp32 PSUM accumulation.  Per-core device
program (identical NEFF on all cores, only the input shards differ):

  1. Per 512-row s-chunk, x[b] is SWDGE-cast f32->bf16 straight into SBUF,
     then 128x128 blocks are transposed on the TensorEngine (bf16
     transpose-mode, identity as the moving operand) into xT [D=8x128, S] --
     TensorE contracts over the partition axis, so activations must be
     D-major.  (The DMA-xbar transpose path is avoided: its mode-switch
     hazard makes Tile serialize every transpose batch against all other
     DMA traffic.)
  2. QKV projections produce qT/kT [256, S] (head-dim on partitions) and
     v1 [S, 260] (natural orientation, with a ones-column per head appended
     via the bias row so the PV matmul also yields softmax denominators).
     The softmax 1/sqrt(D) scale is folded into Wq/bq on the host.
  3. Attention per head pair: logitsT[t, s] tiles from K=64 matmuls with two
     heads packed in the PE array via row tile_position into one 2-bank PSUM
     tile, exp on ScalarE straight out of PSUM (no max-subtraction: logits
     are O(0.5) by construction), causal triangle mask as a bf16 multiply on
     diagonal tiles only, and PV accumulates
     zT_aug[65, s] = [v.T @ expT ; sum_t expT] (row 64 = denominator).
     The t-loop is a rolling software pipeline: QK(t) issues while exp(t-1)
     runs and PV(t-LAG) consumes, and independent projection / out-proj
     units are popped in periodically to fill the in-order PE's exp-wait
     slack.  Normalization is reciprocal + gpsimd partition_broadcast +
     vector multiply.
  4. Each core computes its full-width partial output
     out_partial = z_local @ Wo.T[local 256 rows, :], interleaved into the
     next chunk's attention.  The host sums the 4 partials per batch (the
     row-parallel reduction of the head-sharded out-projection).
"""

import numpy as np

EMBED_DIM = 1024
NUM_HEADS = 16
HEAD_DIM = 64
BATCH = 2
N_CORES = 8
CORES_PER_BATCH = 4
HEADS_PER_CORE = 4
DQ = HEADS_PER_CORE * HEAD_DIM  # 256 q/k/v channels per core
VW = HEAD_DIM + 1  # v block width incl. ones column
DV1 = HEADS_PER_CORE * VW  # 260
P = 128

_NC_CACHE = {}


def _build_nc(seq):
    import concourse.bass as bass  # noqa: F401
    import concourse.mybir as mybir
    import concourse.tile as tile
    from concourse import bacc

    fp32 = mybir.dt.float32
    bf16 = mybir.dt.bfloat16
    AF = mybir.ActivationFunctionType
    ALU = mybir.AluOpType

    S = seq
    SC = 512  # s-chunk width
    NSC = S // SC  # s-chunks
    NT = S // P  # t-tiles
    ND = EMBED_DIM // P  # D-tiles (8)
    TPC = SC // P  # t-tiles per s-chunk (4)

    nc = bacc.Bacc("TRN2", target_bir_lowering=False, num_devices=N_CORES)

    x = nc.declare_dram_parameter("x", [S, EMBED_DIM], fp32, isOutput=False)
    wq = nc.declare_dram_parameter("wq", [EMBED_DIM, DQ], bf16, isOutput=False)
    bq = nc.declare_dram_parameter("bq", [DQ], fp32, isOutput=False)
    wk = nc.declare_dram_parameter("wk", [EMBED_DIM, DQ], bf16, isOutput=False)
    bk = nc.declare_dram_parameter("bk", [DQ], fp32, isOutput=False)
    wv1 = nc.declare_dram_parameter("wv1", [EMBED_DIM, DV1], bf16, isOutput=False)
    bv1 = nc.declare_dram_parameter("bv1", [DV1], bf16, isOutput=False)
    wot = nc.declare_dram_parameter("wot", [DQ, EMBED_DIM], bf16, isOutput=False)
    out = nc.declare_dram_parameter("out", [S, EMBED_DIM], fp32, isOutput=True)

    with tile.TileContext(nc) as tc:
        with (
            tc.tile_pool(name="const", bufs=1) as constp,
            tc.tile_pool(name="big", bufs=1) as big,
            tc.tile_pool(name="exp", bufs=16) as expp,
            tc.tile_pool(name="small", bufs=6) as small,
            tc.tile_pool(name="outsb", bufs=4) as outsb,
            tc.tile_pool(name="xnat", bufs=2) as xnp,
            tc.tile_pool(name="psA", bufs=2, space="PSUM") as psA,
            tc.tile_pool(name="psLG", bufs=2, space="PSUM") as psLG,
            tc.tile_pool(name="psZ", bufs=2, space="PSUM") as psZ,
        ):
            # ---- tiles ----------------------------------------------------
            wq_sb = big.tile([P, ND, DQ], bf16, name="wq_sb")
            wk_sb = big.tile([P, ND, DQ], bf16, name="wk_sb")
            wv1_sb = big.tile([P, ND, DV1], bf16, name="wv1_sb")
            wot_sb = big.tile([P, DQ // P, EMBED_DIM], bf16, name="wot_sb")
            bq_sb = constp.tile([P, DQ // P], fp32, name="bq_sb")
            bk_sb = constp.tile([P, DQ // P], fp32, name="bk_sb")
            bv1_bf = constp.tile([1, DV1], bf16, name="bv1_bf")
            bv1_bc = constp.tile([P, DV1], bf16, name="bv1_bc")
            ones_bf = constp.tile([1, P], bf16, name="ones_bf")
            mask_f = constp.tile([P, P], fp32, name="mask_f")
            mask_bf = constp.tile([P, P], bf16, name="mask_bf")
            ident_f = constp.tile([P, P], fp32, name="ident_f")
            ident_bf = constp.tile([P, P], bf16, name="ident_bf")
            xT = big.tile([P, ND, S], bf16, name="xT")
            qT = big.tile([P, DQ // P, S], bf16, name="qT")
            kT = big.tile([P, DQ // P, S], bf16, name="kT")
            v1 = big.tile([P, NT, DV1], bf16, name="v1")
            # normalized z, two heads stacked per partition tile (for out-proj)
            zT2 = big.tile([P, DQ // P, S], bf16, name="zT2")

            # ---- emission helpers (units = closures emitted round-robin) --
            xn_pending = {}

            def emit_xcast(c):
                # SWDGE-cast x rows straight into SBUF (f32->bf16).  Issued
                # well before the transposes so the in-order PE never waits
                # on the cast DMA.
                xn = xnp.tile([P, TPC, EMBED_DIM], bf16, name="xnat")
                nc.gpsimd.dma_start(
                    xn[:],
                    x[c * SC : (c + 1) * SC, :].rearrange("(o p) n -> p o n", p=P),
                )
                xn_pending[c] = xn

            def emit_xtranspose(c):
                # Transpose 128x128 blocks on the PE (identity stays moving,
                # x block is the stationary operand; bf16 transpose-mode runs
                # at 1 cycle/row).  Avoids the DMA-xbar transpose path, whose
                # mode-switch hazard serializes against all other DMA traffic.
                xn = xn_pending.pop(c)
                for d in range(ND):
                    ps = psA.tile([P, SC], bf16, name="mmps")
                    for tt in range(TPC):
                        nc.tensor.transpose(
                            ps[:, tt * P : (tt + 1) * P],
                            xn[:, tt, d * P : (d + 1) * P],
                            ident_bf[:],
                        )
                    nc.vector.tensor_copy(xT[:, d, c * SC : (c + 1) * SC], ps[:])

            def unit_qk_proj(c, which, j, ps=None, dr=None):
                w_sb, b_sb, dstT = (
                    (wq_sb, bq_sb, qT) if which == "q" else (wk_sb, bk_sb, kT)
                )
                if ps is None:
                    ps = psA.tile([P, SC], fp32, name="mmps")
                for d in dr if dr is not None else range(ND):
                    nc.tensor.matmul(
                        ps[:],
                        w_sb[:, d, j * P : (j + 1) * P],
                        xT[:, d, c * SC : (c + 1) * SC],
                        start=(d == 0),
                        stop=(d == ND - 1),
                    )
                if dr is None or dr[-1] == ND - 1:
                    # bias-add on ScalarE: DVE's in-order queue is busy with
                    # norm/mask work at chunk boundaries and would delay the
                    # next chunk's first QK
                    nc.scalar.activation(
                        dstT[:, j, c * SC : (c + 1) * SC],
                        ps[:],
                        AF.Identity,
                        bias=b_sb[:, j : j + 1],
                    )
                return ps

            def unit_v_proj(c, tt):
                ps = psA.tile([P, SC], fp32, name="mmps")[:, :DV1]
                for d in range(ND):
                    nc.tensor.matmul(
                        ps[:],
                        xT[:, d, tt * P : (tt + 1) * P],
                        wv1_sb[:, d, :],
                        start=(d == 0),
                        stop=(d == ND - 1),
                    )
                nc.vector.tensor_tensor(v1[:, tt, :], ps[:], bv1_bc[:], ALU.add)

            def proj_units(c):
                yield lambda: unit_qk_proj(c, "q", 0)
                yield lambda: unit_qk_proj(c, "k", 0)
                yield lambda: unit_qk_proj(c, "q", 1)
                yield lambda: unit_qk_proj(c, "k", 1)
                for tt in range(c * TPC, (c + 1) * TPC):
                    yield lambda tt=tt: unit_v_proj(c, tt)

            def unit_outproj(c, i):
                # out rows [i*P:(i+1)*P] = z_local.T @ wot  (i is a global s-tile)
                ps = [psA.tile([P, SC], fp32, name="mmps") for _ in range(2)]
                for n in range(2):
                    for j in range(DQ // P):
                        nc.tensor.matmul(
                            ps[n][:],
                            zT2[:, j, i * P : (i + 1) * P],
                            wot_sb[:, j, n * SC : (n + 1) * SC],
                            start=(j == 0),
                            stop=(j == DQ // P - 1),
                        )
                osb = outsb.tile([P, EMBED_DIM], fp32, name="osb")
                for n in range(2):
                    nc.vector.tensor_copy(osb[:, n * SC : (n + 1) * SC], ps[n][:])
                nc.sync.dma_start(out[i * P : (i + 1) * P, :], osb[:])

            def outproj_units(c):
                for i in range(c * TPC, (c + 1) * TPC):
                    yield lambda i=i: unit_outproj(c, i)

            # ---- attention for one (s-chunk, head-pair) ------------------
            def emit_attention(c, hp, filler):
                """Rolling QK -> exp -> (lag-2) PV pipeline; `filler` units are
                popped periodically to fill the PE's exp-wait slack."""
                LAG = 3
                zps = [psZ.tile([VW, SC], fp32, name="zps") for _ in range(2)]
                ntt = (c + 1) * TPC
                exs = {}

                def emit_pv(tt):
                    off = max(0, (tt - c * TPC)) * P
                    ex = exs.pop(tt)
                    for i in range(2):
                        h = 2 * hp + i
                        nc.tensor.matmul(
                            zps[i][:, off:],
                            v1[:, tt, h * VW : (h + 1) * VW],
                            ex[:, i, off:],
                            start=(tt == 0),
                            stop=(tt == ntt - 1),
                        )

                for tt in range(ntt):
                    off = max(0, (tt - c * TPC)) * P
                    lg = psLG.tile([P, 2, SC], fp32, name="lgps")
                    ex = expp.tile([P, 2, SC], bf16, name="expt")
                    exs[tt] = ex
                    for i in range(2):
                        p0 = 64 * i
                        nc.tensor.matmul(
                            lg[:, i, off:],
                            kT[p0 : p0 + 64, hp, tt * P : (tt + 1) * P],
                            qT[p0 : p0 + 64, hp, c * SC + off : (c + 1) * SC],
                            start=True,
                            stop=True,
                            tile_position=(p0, 0),
                        )
                    nc.scalar.activation(ex[:, :, off:], lg[:, :, off:], AF.Exp)
                    if tt >= c * TPC:  # diagonal tile: causal triangle
                        for i in range(2):
                            nc.vector.tensor_tensor(
                                ex[:, i, off : off + P],
                                ex[:, i, off : off + P],
                                mask_bf[:],
                                ALU.mult,
                            )
                    if tt >= LAG:
                        emit_pv(tt - LAG)
                    if tt % 4 == 3:
                        for f in filler:  # fill PE exp-wait slack
                            f()
                            break
                for tt in range(max(0, ntt - LAG), ntt):
                    emit_pv(tt)
                # normalization: z = zT_aug[0:64] * (1 / denom_row)
                for i in range(2):
                    h = 2 * hp + i
                    recip = small.tile([1, SC], fp32, name="recip")
                    rb = small.tile([HEAD_DIM, SC], fp32, name="recip_bc")
                    nc.vector.reciprocal(recip[:], zps[i][VW - 1 : VW, :])
                    nc.gpsimd.partition_broadcast(rb[:], recip[:])
                    if i == 0:  # even head: write partitions 0:64 directly
                        nc.vector.tensor_tensor(
                            zT2[:HEAD_DIM, hp, c * SC : (c + 1) * SC],
                            zps[i][:HEAD_DIM, :],
                            rb[:],
                            ALU.mult,
                        )
                    else:  # odd head: normalize to scratch, DMA-shift partitions
                        zodd = small.tile([HEAD_DIM, SC], bf16, name="zodd")
                        nc.vector.tensor_tensor(
                            zodd[:], zps[i][:HEAD_DIM, :], rb[:], ALU.mult
                        )
                        nc.sync.dma_start(
                            zT2[HEAD_DIM:P, hp, c * SC : (c + 1) * SC], zodd[:]
                        )

            # ---- main schedule -------------------------------------------
            # ---- startup: issue the chunk-0 cast DMA first, generate the
            # constants while it is in flight, then transpose
            xn0 = xnp.tile([P, TPC, EMBED_DIM], bf16, name="xnat")
            nc.gpsimd.dma_start(
                xn0[:], x[:SC, :].rearrange("(o p) n -> p o n", p=P)
            )
            from concourse.masks import make_identity

            make_identity(nc, ident_f[:])
            nc.vector.tensor_copy(ident_bf[:], ident_f[:])
            nc.gpsimd.memset(ones_bf[:], 1.0)
            # causal triangle mask (keep where t_local <= s_local)
            nc.gpsimd.memset(mask_f[:], 0.0)
            nc.gpsimd.affine_select(
                out=mask_f[:],
                in_=mask_f[:],
                compare_op=ALU.is_gt,  # iota > 0 ? keep in_ (0.0) : fill (1.0)
                fill=1.0,
                base=0,
                pattern=[[-1, P]],  # iota[p, f] = p - f;  p<=f -> fill=1.0
                channel_multiplier=1,
            )
            nc.vector.tensor_copy(mask_bf[:], mask_f[:])
            for d in range(ND):
                ps = psA.tile([P, SC], bf16, name="mmps")
                for tt in range(TPC):
                    nc.tensor.transpose(
                        ps[:, tt * P : (tt + 1) * P],
                        xn0[:, tt, d * P : (d + 1) * P],
                        ident_bf[:],
                    )
                nc.vector.tensor_copy(xT[:, d, :SC], ps[:])
            nc.sync.dma_start(wq_sb[:], wq.rearrange("(o p) n -> p o n", p=P))
            nc.sync.dma_start(bq_sb[:], bq.rearrange("(o p) -> p o", p=P))
            nc.sync.dma_start(wk_sb[:], wk.rearrange("(o p) n -> p o n", p=P))
            nc.sync.dma_start(bk_sb[:], bk.rearrange("(o p) -> p o", p=P))
            nc.sync.dma_start(wv1_sb[:], wv1.rearrange("(o p) n -> p o n", p=P))
            nc.sync.dma_start(bv1_bf[:], bv1[None, :])
            nc.gpsimd.partition_broadcast(bv1_bc[:], bv1_bf[:])
            nc.sync.dma_start(wot_sb[:], wot.rearrange("(o p) n -> p o n", p=P))
            # chunk 0: split the first projection into d-halves so the first
            # matmuls only wait for the first half of the x transposes
            ps0 = unit_qk_proj(0, "q", 0, dr=range(ND // 2))
            unit_qk_proj(0, "q", 0, ps=ps0, dr=range(ND // 2, ND))
            unit_qk_proj(0, "k", 0)
            unit_qk_proj(0, "q", 1)
            unit_qk_proj(0, "k", 1)
            for tt in range(TPC):
                unit_v_proj(0, tt)
            outproj_done = set()
            for c in range(NSC):
                # in-group fillers: only always-ready work (prev chunk outproj)
                # later chunks have more attention iterations (more PE
                # exp-wait slack) but fewer natural fillers -- bias the
                # out-proj filler supply toward the final chunk
                if c == NSC - 1:
                    todo = [cc for cc in range(NSC - 1) if cc not in outproj_done]
                elif c >= 2:
                    todo = [c - 2]
                else:
                    todo = []
                outproj_done.update(todo)
                pending = [u for cc in todo for u in outproj_units(cc)]
                filler = iter(pending)
                if c + 1 < NSC:
                    emit_xcast(c + 1)
                emit_attention(c, 0, filler)
                if c + 1 < NSC:
                    emit_xtranspose(c + 1)
                emit_attention(c, 1, filler)
                for f in filler:
                    f()
                # then next chunk's projections
                if c + 1 < NSC:
                    for u in proj_units(c + 1):
                        u()
            for u in outproj_units(NSC - 1):
                u()

    nc.finalize()
    return nc


def _get_nc(seq):
    if seq not in _NC_CACHE:
        _NC_CACHE[seq] = _build_nc(seq)
    return _NC_CACHE[seq]


def shard_inputs(x, Wq, bq, Wk, bk, Wv, bv, Wo):
    """Build the 8 per-core input maps (host-side sharding)."""
    import ml_dtypes

    bf = ml_dtypes.bfloat16
    scale = 1.0 / np.sqrt(np.float32(EMBED_DIM))
    x = np.asarray(x, np.float32)
    in_maps = []
    for c in range(N_CORES):
        b, g = divmod(c, CORES_PER_BATCH)
        sl = slice(g * DQ, (g + 1) * DQ)
        wv1 = np.zeros((EMBED_DIM, DV1), np.float32)
        bv1 = np.zeros((DV1,), np.float32)
        for h in range(HEADS_PER_CORE):
            col = g * DQ + h * HEAD_DIM
            wv1[:, h * VW : h * VW + HEAD_DIM] = Wv[:, col : col + HEAD_DIM]
            bv1[h * VW : h * VW + HEAD_DIM] = bv[col : col + HEAD_DIM]
            bv1[h * VW + HEAD_DIM] = 1.0
        in_maps.append(
            {
                "x": np.ascontiguousarray(x[b]),
                "wq": (np.ascontiguousarray(Wq[:, sl]) * scale).astype(bf),
                "bq": np.ascontiguousarray(bq[sl]) * scale,
                "wk": np.ascontiguousarray(Wk[:, sl]).astype(bf),
                "bk": np.ascontiguousarray(bk[sl]),
                "wv1": wv1.astype(bf),
                "bv1": bv1.astype(bf),
                "wot": np.ascontiguousarray(Wo[:, sl].T).astype(bf),
            }
        )
    return in_maps


def kernel(x, Wq, bq, Wk, bk, Wv, bv, Wo):
    from concourse.bass_utils import run_bass_kernel_spmd

    x = np.asarray(x, np.float32)
    B, S, D = x.shape
    nc = _get_nc(S)
    in_maps = shard_inputs(x, Wq, bq, Wk, bk, Wv, bv, Wo)
    res = run_bass_kernel_spmd(nc, in_maps, core_ids=list(range(N_CORES)))
    out = np.zeros((B, S, D), np.float32)
    for c in range(N_CORES):
        b = c // CORES_PER_BATCH
        out[b] += res.results[c]["out"]
    return out
